# revision 10
# baseline (speedup 1.0000x reference)
"""WaveNet convolution stack (20 dilated layers) on 8 Trainium2 NeuronCores.

Sharding: batch(4) x time(2). Each core processes one batch element and one
time half. The right time-half recomputes a 2048-column causal halo
(receptive field of the 20-layer stack is sum(d*(K-1)) = 2046), so cores are
fully independent SPMD - no collectives.

Per-core kernel: the residual stream x ping-pongs between two fp16 SBUF
buffers [128, 512+5120] whose first 512 columns are causal zero-padding. All
weights stay resident in SBUF (host pre-transposes them into matmul lhsT
layout, fp16). A dilated conv is two accumulated fp16 matmuls reading the
same SBUF buffer at column offsets (t-d) and (t): the time shift costs no
data movement. Gating runs on ScalarE (tanh/sigmoid with the conv bias fused
into the activation, fp16 out), the gate multiply and output bias on VectorE,
the residual add alternates VectorE/GpSimd for engine balance, and each
layer's skip tile is DMA'd to HBM as fp16 (host widens to fp32).
"""

import sys

import numpy as np

for _p in ("/opt/trn_rl_repo",):
    if _p not in sys.path:
        sys.path.insert(0, _p)

DILATIONS = [1, 2, 4, 8, 16, 32, 64, 128, 256, 512] * 2
L = 20
C = 128
B = 4
T = 8192

PAD = 512          # max dilation: causal zero-pad columns
T_COMP = 5120      # columns each core computes
TBUF = PAD + T_COMP
TW = 1024          # elementwise tile width (2 PSUM banks; matmuls do 512)
MW = 512           # matmul free-dim (one fp32 PSUM bank)
NT = T_COMP // TW
HALO = 2048        # right-half cores discard their first HALO computed cols
START1 = T - T_COMP  # 3072: global start of the right half's compute region

_CACHED = None  # compiled Bass module - build once per process


def _build():
    from concourse import bacc, mybir, tile

    f32 = mybir.dt.float32
    f16 = mybir.dt.float16
    AF = mybir.ActivationFunctionType

    nc = bacc.Bacc("TRN2", target_bir_lowering=False)

    x_in = nc.dram_tensor("x_in", [C, T_COMP], f16, kind="ExternalInput")
    # [l, c_in, tap, half, c_out]: lhsT layout, contiguous per (l, c_in)
    wc_in = nc.dram_tensor("wc_in", [L, C, 2, 2, C], f16, kind="ExternalInput")
    # [l, c_in, c_out]
    wo_in = nc.dram_tensor("wo_in", [L, C, C], f16, kind="ExternalInput")
    # [c, l, 3]: tanh-bias, sigmoid-bias, out-bias
    b_in = nc.dram_tensor("b_in", [C, L, 3], f32, kind="ExternalInput")

    skips_out = nc.dram_tensor("skips", [L, C, T_COMP], f16, kind="ExternalOutput")
    x_out = nc.dram_tensor("x_out", [C, T_COMP], f16, kind="ExternalOutput")

    with tile.TileContext(nc) as tc:
        with (
            tc.tile_pool(name="res", bufs=1) as res,
            tc.tile_pool(name="work", bufs=6) as work,
            tc.tile_pool(name="psum", bufs=4, space="PSUM") as ps,
        ):
            xa = res.tile([C, TBUF], f16, tag="xa")
            xb = res.tile([C, TBUF], f16, tag="xb")
            wc = res.tile([C, L, 2, 2, C], f16, tag="wc")
            wo = res.tile([C, L, C], f16, tag="wo")
            bias = res.tile([C, L, 3], f32, tag="bias")

            nc.vector.memset(xa[:, :PAD], 0.0)
            nc.vector.memset(xb[:, :PAD], 0.0)
            nc.sync.dma_start(xa[:, PAD:], x_in[:])
            nc.sync.dma_start(bias[:], b_in[:])
            for l in range(L):
                nc.sync.dma_start(wc[:, l], wc_in[l])
                nc.sync.dma_start(wo[:, l], wo_in[l])

            bufs = [xa, xb]
            # Software pipeline: the out-conv matmul for stage s-1 is issued
            # after stage s's conv matmuls, so the PE never waits on the
            # MM -> ACT -> mul gate chain. `pending` carries (l, t, g) one
            # stage behind, across layer boundaries.
            pending = None

            def flush(pending):
                pl, pt, pg = pending
                poff = PAD + pt * TW
                pxc = bufs[pl % 2]
                pxn = bufs[(pl + 1) % 2]
                sp = ps.tile([C, TW], f32, tag="z")
                nc.tensor.matmul(
                    sp[:, :MW], wo[:, pl], pg[:, :MW], start=True, stop=True
                )
                nc.tensor.matmul(
                    sp[:, MW:], wo[:, pl], pg[:, MW:], start=True, stop=True
                )
                sk = work.tile([C, TW], f16, tag="sk")
                nc.vector.tensor_scalar_add(sk, sp, bias[:, pl, 2:3])
                # residual add: split GpSimd / VectorE so neither engine
                # becomes the bottleneck
                H = TW // 2
                nc.gpsimd.tensor_add(
                    out=pxn[:, poff : poff + H],
                    in0=pxc[:, poff : poff + H],
                    in1=sk[:, :H],
                )
                nc.vector.tensor_add(
                    out=pxn[:, poff + H : poff + TW],
                    in0=pxc[:, poff + H : poff + TW],
                    in1=sk[:, H:],
                )
                nc.sync.dma_start(
                    skips_out[pl, :, pt * TW : (pt + 1) * TW], sk
                )

            for l in range(L):
                d = DILATIONS[l]
                xc = bufs[l % 2]
                for t in range(NT):
                    off = PAD + t * TW
                    rhs_cur = xc[:, off : off + TW]
                    rhs_past = xc[:, off - d : off - d + TW]

                    zt = ps.tile([C, TW], f32, tag="z")
                    zs = ps.tile([C, TW], f32, tag="z")
                    for half, zz in ((0, zt), (1, zs)):
                        for s0 in (0, MW):
                            nc.tensor.matmul(
                                zz[:, s0 : s0 + MW],
                                wc[:, l, 0, half],
                                rhs_past[:, s0 : s0 + MW],
                                start=True, stop=False,
                            )
                            nc.tensor.matmul(
                                zz[:, s0 : s0 + MW],
                                wc[:, l, 1, half],
                                rhs_cur[:, s0 : s0 + MW],
                                start=False, stop=True,
                            )

                    th = work.tile([C, TW], f16, tag="th")
                    sg = work.tile([C, TW], f16, tag="sg")
                    nc.scalar.activation(th, zt, AF.Tanh, bias=bias[:, l, 0:1])
                    nc.scalar.activation(sg, zs, AF.Sigmoid, bias=bias[:, l, 1:2])

                    g = work.tile([C, TW], f16, tag="g")
                    nc.vector.tensor_mul(out=g, in0=th, in1=sg)

                    if pending is not None:
                        flush(pending)
                    pending = (l, t, g)

            flush(pending)
            nc.sync.dma_start(x_out[:], bufs[L % 2][:, PAD:])

    nc.compile()
    return nc


def _get_nc():
    global _CACHED
    if _CACHED is None:
        _CACHED = _build()
    return _CACHED


def _prep_inputs(x, w_conv, b_conv, w_out, b_out):
    x = np.asarray(x, np.float32)
    w_conv = np.asarray(w_conv, np.float32)
    b_conv = np.asarray(b_conv, np.float32)
    w_out = np.asarray(w_out, np.float32)
    b_out = np.asarray(b_out, np.float32)

    # [L, 2C, C, K] -> [L, C(c_in), K(tap), 2(half), C(c_out)]
    wc_host = np.ascontiguousarray(
        w_conv.transpose(0, 2, 3, 1).reshape(L, C, 2, 2, C)
    ).astype(np.float16)
    wo_host = np.ascontiguousarray(w_out.transpose(0, 2, 1)).astype(np.float16)
    bias_host = np.ascontiguousarray(
        np.stack([b_conv[:, :C].T, b_conv[:, C:].T, b_out.T], axis=2)
    )

    in_maps = []
    for b in range(B):
        for half in range(2):
            start = 0 if half == 0 else START1
            in_maps.append(
                {
                    "x_in": np.ascontiguousarray(
                        x[b, :, start : start + T_COMP]
                    ).astype(np.float16),
                    "wc_in": wc_host,
                    "wo_in": wo_host,
                    "b_in": bias_host,
                }
            )
    return in_maps


def _gather(results):
    x_full = np.empty((B, C, T), np.float32)
    skips_full = np.empty((L, B, C, T), np.float32)
    for b in range(B):
        for half in range(2):
            r = results[b * 2 + half]
            if half == 0:
                x_full[b, :, :T_COMP] = r["x_out"]
                skips_full[:, b, :, :T_COMP] = r["skips"]
            else:
                x_full[b, :, T_COMP:] = r["x_out"][:, HALO:]
                skips_full[:, b, :, T_COMP:] = r["skips"][:, :, HALO:]
    return x_full, skips_full


def kernel(x, w_conv, b_conv, w_out, b_out, _trace=False):
    from concourse import bass_utils

    nc = _get_nc()
    in_maps = _prep_inputs(x, w_conv, b_conv, w_out, b_out)
    out = bass_utils.run_bass_kernel_spmd(
        nc, in_maps, core_ids=list(range(8)), trace=_trace
    )
    x_full, skips_full = _gather(out.results)
    if _trace:
        kernel.last_result = out
    return x_full, skips_full


# revision 12
# speedup vs baseline: 1.1116x; 1.1116x over previous
"""WaveNet convolution stack (20 dilated layers) on 8 Trainium2 NeuronCores.

Sharding: batch(4) x time(2). Each core processes one batch element and one
time half. The right time-half recomputes a 2048-column causal halo
(receptive field of the 20-layer stack is sum(d*(K-1)) = 2046), so cores are
fully independent SPMD - no collectives.

Per-core kernel: the residual stream x lives in ONE fp16 SBUF buffer
[128, 512+5120] whose first 512 columns are causal zero-padding; layers
update it in place. All weights stay resident in SBUF (host pre-transposes
them into matmul lhsT layout, fp16). A dilated conv is two accumulated fp16
matmuls reading the same buffer at column offsets (t-d) and (t): the time
shift costs no data movement. Per 512-column tile: gating runs on ScalarE
(tanh/sigmoid with the conv bias fused into the activation, fp16 out), the
gate multiply and output bias on VectorE, and the residual add rides a
software-DGE accumulate-DMA (accum_op=add) so it costs no compute-engine
time. In-place update is safe because dilation <= tile width: only tile t+1
reads tile t's pre-update values, and the out-conv for tile t is issued one
stage behind (software pipeline), so Tile's WAR tracking orders the
accumulate after those reads. Skips are DMA'd to HBM as fp16 (host widens).
"""

import sys

import numpy as np

for _p in ("/opt/trn_rl_repo",):
    if _p not in sys.path:
        sys.path.insert(0, _p)

DILATIONS = [1, 2, 4, 8, 16, 32, 64, 128, 256, 512] * 2
L = 20
C = 128
B = 4
T = 8192

PAD = 512          # max dilation: causal zero-pad columns
T_COMP = 5120      # columns each core computes
TBUF = PAD + T_COMP
TW = 512           # matmul free-dim tile (one PSUM bank of fp32)
NT = T_COMP // TW
HALO = 2048        # right-half cores discard their first HALO computed cols
START1 = T - T_COMP  # 3072: global start of the right half's compute region

_CACHED = None  # compiled Bass module - build once per process


def _build():
    from concourse import bacc, mybir, tile

    f32 = mybir.dt.float32
    f16 = mybir.dt.float16
    AF = mybir.ActivationFunctionType
    ADD = mybir.AluOpType.add

    nc = bacc.Bacc("TRN2", target_bir_lowering=False, num_swdge_queues=4)

    x_in = nc.dram_tensor("x_in", [C, T_COMP], f16, kind="ExternalInput")
    # [l, c_in, tap, half, c_out]: lhsT layout, contiguous per (l, c_in)
    wc_in = nc.dram_tensor("wc_in", [L, C, 2, 2, C], f16, kind="ExternalInput")
    # [l, c_in, c_out]
    wo_in = nc.dram_tensor("wo_in", [L, C, C], f16, kind="ExternalInput")
    # [c, l, 3]: tanh-bias, sigmoid-bias, out-bias
    b_in = nc.dram_tensor("b_in", [C, L, 3], f32, kind="ExternalInput")

    skips_out = nc.dram_tensor("skips", [L, C, T_COMP], f16, kind="ExternalOutput")
    x_out = nc.dram_tensor("x_out", [C, T_COMP], f16, kind="ExternalOutput")

    with tile.TileContext(nc) as tc:
        with (
            tc.tile_pool(name="res", bufs=1) as res,
            tc.tile_pool(name="work", bufs=6) as work,
            tc.tile_pool(name="psum", bufs=3, space="PSUM") as ps,
            tc.tile_pool(name="psum_s", bufs=2, space="PSUM") as ps_s,
        ):
            x = res.tile([C, TBUF], f16, tag="x")
            wc = res.tile([C, L, 2, 2, C], f16, tag="wc")
            wo = res.tile([C, L, C], f16, tag="wo")
            bias = res.tile([C, L, 3], f32, tag="bias")

            nc.vector.memset(x[:, :PAD], 0.0)
            nc.sync.dma_start(bias[:], b_in[:])
            for t in range(NT):
                nc.sync.dma_start(
                    x[:, PAD + t * TW : PAD + (t + 1) * TW],
                    x_in[:, t * TW : (t + 1) * TW],
                )
            for l in range(L):
                nc.sync.dma_start(wc[:, l], wc_in[l])
                nc.sync.dma_start(wo[:, l], wo_in[l])

            # Software pipeline: the out-conv matmul for stage s-1 is issued
            # after stage s's conv matmuls, so the PE never waits on the
            # MM -> ACT -> mul gate chain. `pending` carries (l, t, g) one
            # stage behind, across layer boundaries.
            pending = None

            def flush(pending):
                pl, pt, pg = pending
                poff = PAD + pt * TW
                sp = ps_s.tile([C, TW], f32, tag="sp")
                nc.tensor.matmul(sp, wo[:, pl], pg, start=True, stop=True)
                sk = work.tile([C, TW], f16, tag="sk")
                nc.vector.tensor_scalar_add(sk, sp, bias[:, pl, 2:3])
                # residual: in-place accumulate via software-DGE DMA; Tile's
                # WAR tracking orders this after all conv reads of the old
                # x values (only tiles pt and pt+1 of layer pl read them)
                nc.gpsimd.dma_start(
                    x[:, poff : poff + TW], sk, accum_op=ADD
                )
                nc.sync.dma_start(
                    skips_out[pl, :, pt * TW : (pt + 1) * TW], sk
                )
                if pl == L - 1:
                    nc.sync.dma_start(
                        x_out[:, pt * TW : (pt + 1) * TW],
                        x[:, poff : poff + TW],
                    )

            for l in range(L):
                d = DILATIONS[l]
                for t in range(NT):
                    off = PAD + t * TW
                    rhs_cur = x[:, off : off + TW]
                    rhs_past = x[:, off - d : off - d + TW]

                    zt = ps.tile([C, TW], f32, tag="zt")
                    zs = ps.tile([C, TW], f32, tag="zs")
                    nc.tensor.matmul(
                        zt, wc[:, l, 0, 0], rhs_past, start=True, stop=False
                    )
                    nc.tensor.matmul(
                        zt, wc[:, l, 1, 0], rhs_cur, start=False, stop=True
                    )
                    nc.tensor.matmul(
                        zs, wc[:, l, 0, 1], rhs_past, start=True, stop=False
                    )
                    nc.tensor.matmul(
                        zs, wc[:, l, 1, 1], rhs_cur, start=False, stop=True
                    )

                    th = work.tile([C, TW], f16, tag="th")
                    sg = work.tile([C, TW], f16, tag="sg")
                    nc.scalar.activation(th, zt, AF.Tanh, bias=bias[:, l, 0:1])
                    nc.scalar.activation(sg, zs, AF.Sigmoid, bias=bias[:, l, 1:2])

                    g = work.tile([C, TW], f16, tag="g")
                    nc.vector.tensor_mul(out=g, in0=th, in1=sg)

                    if pending is not None:
                        flush(pending)
                    pending = (l, t, g)

            flush(pending)

    nc.compile()
    return nc


def _get_nc():
    global _CACHED
    if _CACHED is None:
        _CACHED = _build()
    return _CACHED


def _prep_inputs(x, w_conv, b_conv, w_out, b_out):
    x = np.asarray(x, np.float32)
    w_conv = np.asarray(w_conv, np.float32)
    b_conv = np.asarray(b_conv, np.float32)
    w_out = np.asarray(w_out, np.float32)
    b_out = np.asarray(b_out, np.float32)

    # [L, 2C, C, K] -> [L, C(c_in), K(tap), 2(half), C(c_out)]
    wc_host = np.ascontiguousarray(
        w_conv.transpose(0, 2, 3, 1).reshape(L, C, 2, 2, C)
    ).astype(np.float16)
    wo_host = np.ascontiguousarray(w_out.transpose(0, 2, 1)).astype(np.float16)
    bias_host = np.ascontiguousarray(
        np.stack([b_conv[:, :C].T, b_conv[:, C:].T, b_out.T], axis=2)
    )

    in_maps = []
    for b in range(B):
        for half in range(2):
            start = 0 if half == 0 else START1
            in_maps.append(
                {
                    "x_in": np.ascontiguousarray(
                        x[b, :, start : start + T_COMP]
                    ).astype(np.float16),
                    "wc_in": wc_host,
                    "wo_in": wo_host,
                    "b_in": bias_host,
                }
            )
    return in_maps


def _gather(results):
    x_full = np.empty((B, C, T), np.float32)
    skips_full = np.empty((L, B, C, T), np.float32)
    for b in range(B):
        for half in range(2):
            r = results[b * 2 + half]
            if half == 0:
                x_full[b, :, :T_COMP] = r["x_out"]
                skips_full[:, b, :, :T_COMP] = r["skips"]
            else:
                x_full[b, :, T_COMP:] = r["x_out"][:, HALO:]
                skips_full[:, b, :, T_COMP:] = r["skips"][:, :, HALO:]
    return x_full, skips_full


def kernel(x, w_conv, b_conv, w_out, b_out, _trace=False):
    from concourse import bass_utils

    nc = _get_nc()
    in_maps = _prep_inputs(x, w_conv, b_conv, w_out, b_out)
    out = bass_utils.run_bass_kernel_spmd(
        nc, in_maps, core_ids=list(range(8)), trace=_trace
    )
    x_full, skips_full = _gather(out.results)
    if _trace:
        kernel.last_result = out
    return x_full, skips_full
